# revision 42
# baseline (speedup 1.0000x reference)
"""Trainium2 Bass kernel for sliding-window causal attention block.

Reference computation (per batch b):
  qh = (q @ wq.T)  -> [S, H, Dh], RoPE'd; kh likewise; vh = v @ wv.T
  scores = qh . kh / sqrt(Dh), sliding-window causal (j in (i-512, i])
  out = softmax(scores) @ vh  -> [S, H*Dh] @ wo.T -> [S, D]

Sharding: 8 cores = 2 batches x 4 head-groups (4 heads each).
Each core computes y_part[b] = attn(heads g) @ wo[:, g].T  (f16 partial,
x256 prescale from the fp8 weight splits); the host undoes the scale,
sums the 4 partials per batch and casts to f16.

Key optimizations over the f16 baseline (cost-model driven):
  - All three input projections run as 3-term fp8 DoubleRow matmuls:
    x = xh + xl, w*16 = wh + wl (e4m3 hi/lo splits done on host), and
    x@w*16 ~= xh@wh + xl@wh + xh@wl.  DoubleRow contracts two 128-deep
    k-tiles per instruction at 0.5 cycles/row, so the three terms cost
    0.75x the f16 projection while keeping ~f16 accuracy (the dropped
    xl@wl term is ~0.4% of the result).  Host stores x as [2, D, S]
    (hi, lo) planes; the correction terms pair over kc-chunks so no
    cross-plane slicing is needed.  Weights are packed with kc-pair rows
    in 512-byte lines so their DMA hits full-width descriptors.
  - RoPE head-dim permutation is 16-granular ([e0..15][o0..15][e16..31]
    [o16..31] per head) so the rotary partner lives +-16 partitions away
    within a 32-partition quadrant: the partner swap is a single DVE
    stream_shuffle straight out of PSUM, eliminating the permutation
    matmul.  qT = ps*A + shuffle(ps)*B with A/B f16 tables.
  - Window masks: PE seeds boundary-score PSUM chunks with an additive
    -57600 mask via one fp8 DoubleRow matmul (64 cycles) per boundary
    chunk; the chunk's f16 QK matmul accumulates on top (start=False).
  - Scores stay f16 (fp8 QK fails the 2e-2 gate); exp folds the x256
    weight prescale into its scale and a -3 bias keeps p in f16 range
    without max-subtraction (|score| <= 8*sqrt(Dh) by Cauchy-Schwarz).
  - attn -> attnT uses the DMA transpose XBAR for pipelined tiles (no
    PE/PSUM involved); the 4 tail tiles instead use PE transposes into a
    spare B-phase PSUM bank, dodging the ~2.4us XBAR init+sem latency on
    their critical path.
  - The out-projection is a 3-term fp8 DoubleRow product (attnT split
    hi/lo on the Pool engine, wo*16 split hi/lo on host): 0.75x the f16
    cost.
  - GPSIMD cannot access PSUM on hardware, so DVE owns all PSUM reads
    (RoPE t1, shuffle, v/y casts, normalize); Pool gets the SBUF-only
    multiplies/adds and the k-tensor DMA queue; ACT runs exp plus early
    consts and a share of tail y-copies.  DMA transfer time is charged
    to the issuing engine's sequencer (~0.39 ns per byte-per-partition),
    so the big input loads are spread across the SP/Pool/ACT queues.
  - Projections and attention are software-pipelined: each chunk's
    projection pieces interleave with the previous chunk's attention
    tiles so exp (ACT) and the Tensor engine stay co-scheduled; x tiles
    prefetch two chunks ahead; the 4 tail tiles get a double-buffered
    PV accumulator from the freed projection PSUM banks.
  Cost-model engine busy: PE ~67us, ACT ~58us, DVE ~58us, SP ~41us,
  Pool ~33us; makespan ~85.4us (vs 127.5us f16 baseline), verified on
  hardware at rel err 0.0019.
"""

import os
import sys

import numpy as np
import ml_dtypes

for _p in ("/opt/trn_rl_repo", "/root/.axon_site/_ro/trn_rl_repo"):
    if os.path.isdir(_p) and _p not in sys.path:
        sys.path.insert(0, _p)

F8NP = ml_dtypes.float8_e4m3

DIM = 1024
NUM_HEADS = 16
HEAD_DIM = 64
WINDOW = 512
S = 2048
B = 2
HPC = 4  # heads per core
E = HPC * HEAD_DIM  # 256 = per-core hidden slice
N_CORES = 8
ST = S // 128  # 16 query tiles of 128
KC = DIM // 128  # 8 contraction chunks for projections
WSC = 16.0  # weight prescale for the fp8 hi/lo split
MASKV = -240.0  # fp8 mask value; seeded as diag(240) @ mask = -57600
SEEDC = 240.0
EXP_SCALE = 1.0 / (8.0 * WSC * WSC)  # folds 1/sqrt(Dh) and the x256 prescale
EXP_BIAS = -3.0


def _split8(x):
    hi = np.asarray(x, dtype=np.float32).astype(F8NP)
    lo = (np.asarray(x, dtype=np.float32) - hi.astype(np.float32)).astype(F8NP)
    return hi, lo


def _pack2(xT):
    """[D, N] f32 -> [2, D, N] fp8 laid out (hi, lo)."""
    hi, lo = _split8(xT)
    out = np.empty((2, xT.shape[0], xT.shape[1]), dtype=F8NP)
    out[0] = hi
    out[1] = lo
    return out


def _packw(wT):
    """[D, E] f32 -> [2, KC/2*128, 2E] fp8: kc-pair rows packed to 512B lines
    so the weight DMA hits full-width descriptors."""
    p2 = _pack2(wT)  # [2, D, E]
    out = p2.reshape(2, KC // 2, 2, 128, E).transpose(0, 1, 3, 2, 4)
    return np.ascontiguousarray(out.reshape(2, KC // 2 * 128, 2 * E))


def _head_perm():
    # within each head: [even f<16][odd f<16][even f>=16][odd f>=16] so the
    # rotary partner is +-16 partitions inside a 32-partition quadrant
    p = np.empty(E, dtype=np.int64)
    for h in range(HPC):
        base = h * HEAD_DIM
        p[base + 0 : base + 16] = base + np.arange(0, 32, 2)
        p[base + 16 : base + 32] = base + np.arange(1, 32, 2)
        p[base + 32 : base + 48] = base + np.arange(32, 64, 2)
        p[base + 48 : base + 64] = base + np.arange(33, 64, 2)
    return p


def _rope_tables():
    # A/B factor tables in the 16-granular RoPE layout, [128, S] f16.
    f = np.arange(32, dtype=np.float64)
    inv_freq = 1.0 / (10000.0 ** (2.0 * f / HEAD_DIM))  # [32]
    ang = np.arange(S, dtype=np.float64)[None, :] * inv_freq[:, None]  # [32, S]
    cos = np.cos(ang)
    sin = np.sin(ang)
    A = np.empty((128, S), dtype=np.float64)
    Bt = np.empty((128, S), dtype=np.float64)
    for blk in range(2):  # two 64-partition head blocks per tile
        o = blk * 64
        A[o + 0 : o + 16] = cos[0:16]
        A[o + 16 : o + 32] = cos[0:16]
        A[o + 32 : o + 48] = cos[16:32]
        A[o + 48 : o + 64] = cos[16:32]
        Bt[o + 0 : o + 16] = -sin[0:16]
        Bt[o + 16 : o + 32] = sin[0:16]
        Bt[o + 32 : o + 48] = -sin[16:32]
        Bt[o + 48 : o + 64] = sin[16:32]
    return A.astype(np.float16), Bt.astype(np.float16)


def _consts():
    A, Bt = _rope_tables()
    j = np.arange(128)[:, None]
    i = np.arange(128)[None, :]
    # transposed ([j, i]) additive masks, fp8: diag chunk valid iff i >= j
    maskD = np.where(i >= j, 0.0, MASKV).astype(np.float32)
    maskL = np.where(j > i, 0.0, MASKV).astype(np.float32)
    seedI = np.zeros((128, 2, 128), dtype=F8NP)
    seedI[:, 0, :] = np.eye(128, dtype=np.float32) * SEEDC
    maskDP = np.zeros((128, 2, 128), dtype=F8NP)
    maskDP[:, 0, :] = maskD
    maskLP = np.zeros((128, 2, 128), dtype=F8NP)
    maskLP[:, 0, :] = maskL
    return {
        "ropeA": A,
        "ropeB": Bt,
        "seedI": seedI,
        "maskDP": maskDP,
        "maskLP": maskLP,
        "ident": np.eye(128, dtype=np.float16),
    }


def build_bass(do_compile=True):
    import concourse.bacc as bacc
    import concourse.mybir as mybir
    import concourse.tile as tile
    from concourse.tile import add_dep_helper

    f16 = mybir.dt.float16
    f32 = mybir.dt.float32
    fp8 = mybir.dt.float8e4
    DR = mybir.MatmulPerfMode.DoubleRow
    Exp = mybir.ActivationFunctionType.Exp

    nc = bacc.Bacc("TRN2")

    xq = nc.dram_tensor("xq", [2, DIM, S], fp8, kind="ExternalInput")
    xk = nc.dram_tensor("xk", [2, DIM, S], fp8, kind="ExternalInput")
    xv = nc.dram_tensor("xv", [2, DIM, S], fp8, kind="ExternalInput")
    wq = nc.dram_tensor("wq", [2, KC // 2 * 128, 2 * E], fp8, kind="ExternalInput")
    wk = nc.dram_tensor("wk", [2, KC // 2 * 128, 2 * E], fp8, kind="ExternalInput")
    wv = nc.dram_tensor("wv", [2, KC // 2 * 128, 2 * E], fp8, kind="ExternalInput")
    woT = nc.dram_tensor("woT", [E, DIM], f16, kind="ExternalInput")
    woP = nc.dram_tensor("woP", [2, E, DIM], fp8, kind="ExternalInput")
    ident = nc.dram_tensor("ident", [128, 128], f16, kind="ExternalInput")
    ropeA = nc.dram_tensor("ropeA", [128, S], f16, kind="ExternalInput")
    ropeB = nc.dram_tensor("ropeB", [128, S], f16, kind="ExternalInput")
    seedI = nc.dram_tensor("seedI", [128, 2, 128], fp8, kind="ExternalInput")
    maskDP = nc.dram_tensor("maskDP", [128, 2, 128], fp8, kind="ExternalInput")
    maskLP = nc.dram_tensor("maskLP", [128, 2, 128], fp8, kind="ExternalInput")
    y = nc.dram_tensor("y", [S, DIM], f16, kind="ExternalOutput")

    SHUF = [(i + 16) % 32 for i in range(32)]

    with tile.TileContext(nc) as tc:
        # PSUM is 8 banks of 2KB: pp 2 (proj, q/k/v share the "ps" tag) +
        # pst 4 (scores, double-buffered) + po 1 + py 1.
        with tc.tile_pool(name="res", bufs=1) as res, \
             tc.tile_pool(name="xp", bufs=3) as xp, \
             tc.tile_pool(name="rp", bufs=3) as rp, \
             tc.tile_pool(name="sb2", bufs=2) as sb2:
            # resident tensors
            qT = res.tile([128, 2, S], f16)
            kT = res.tile([128, 2, S], f16)
            v_sb = res.tile([128, ST, HPC, 65], f16)
            woT_sb = res.tile([128, 2, DIM], f16)
            woP_sb = res.tile([128, 2, 2, DIM], fp8)
            wq_sb = res.tile([128, 2, KC // 2, 2 * E], fp8)
            wk_sb = res.tile([128, 2, KC // 2, 2 * E], fp8)
            wv_sb = res.tile([128, 2, KC // 2, 2 * E], fp8)
            A_sb = res.tile([128, S], f16)
            B_sb = res.tile([128, S], f16)
            seedI_sb = res.tile([128, 2, 128], fp8)
            maskDP_sb = res.tile([128, 2, 128], fp8)
            maskLP_sb = res.tile([128, 2, 128], fp8)
            bias_sb = res.tile([128, 1], f32)
            ident_sb = res.tile([128, 128], f16)

            W_ENG = {id(wq): (wq, wq_sb), id(wk): (wk, wk_sb), id(wv): (wv, wv_sb)}

            def load_w(wt, plane, eng):
                dram, sb = W_ENG[id(wt)]
                eng.dma_start(
                    out=sb[:, plane],
                    in_=dram[plane].rearrange("(c p) s -> p c s", p=128),
                )

            load_w(wq, 0, nc.sync)
            load_w(wk, 0, nc.gpsimd)
            load_w(wv, 0, nc.scalar)
            nc.any.memset(bias_sb, EXP_BIAS)
            nc.any.memset(v_sb[:, :, :, 64:65], 1.0)

            def emit_dma(sc):
                ssl = slice(sc * 512, (sc + 1) * 512)
                xt = {}
                for name, dram, eng in (("q", xq, nc.sync), ("k", xk, nc.gpsimd),
                                        ("v", xv, nc.sync)):
                    t = xp.tile([128, 2, KC, 512], fp8, tag=f"x{name}")
                    xt[name] = t
                def xdma(name, dram, eng, plane, half=None):
                    if half is None:
                        eng.dma_start(
                            out=xt[name][:, plane],
                            in_=dram[plane, :, ssl].rearrange("(c p) s -> p c s", p=128),
                        )
                    else:
                        ksl = slice(half * 4, half * 4 + 4)
                        eng.dma_start(
                            out=xt[name][:, plane, ksl],
                            in_=dram[plane, half * 512 : half * 512 + 512, ssl]
                                .rearrange("(c p) s -> p c s", p=128),
                        )

                xdma("q", xq, nc.sync, 0)
                xdma("k", xk, nc.gpsimd, 0)
                if sc == 0:
                    load_w(wq, 1, nc.scalar)
                    load_w(wk, 1, nc.gpsimd)
                    load_w(wv, 1, nc.scalar)
                    xdma("v", xv, nc.scalar, 0)
                xdma("q", xq, nc.sync, 1)
                xdma("k", xk, nc.gpsimd, 1)
                if sc == 0:
                    nc.scalar.dma_start(out=A_sb, in_=ropeA[:])
                    nc.scalar.dma_start(out=B_sb, in_=ropeB[:])
                else:
                    xdma("v", xv, nc.sync, 0)
                xdma("v", xv, nc.sync, 1)
                return xt

            def emit_proj_qk(sc, xt, which):
                """q/k projection + RoPE units for s-chunk sc.  which is a
                tuple of tensor names; units interleave ec-outer so a stall
                on one tensor's x-lo DMA overlaps the other's main terms."""

                def mm3(ps, pcol, w_sb, x_t, esl, scol, n):
                    # 3-term hi/lo DoubleRow projection chunk (n out cols):
                    # main: sum over kc pairs of xh.T@wh; corr: xl@wh + xh@wl
                    out = ps[:, pcol : pcol + n]
                    terms = ((0, 0), (1, 0), (0, 1))  # (w plane, x plane)
                    for ti, (wp, xp_) in enumerate(terms):
                        for c in range(KC // 2):
                            wpair = w_sb[:, wp, c].rearrange("p (u e) -> p u e", u=2)
                            nc.tensor.matmul(
                                out,
                                lhsT=wpair[:, :, esl],
                                rhs=x_t[:, xp_, 2 * c : 2 * c + 2, scol : scol + n],
                                start=(ti == 0 and c == 0),
                                stop=(ti == 2 and c == KC // 2 - 1),
                                perf_mode=DR,
                            )

                # q/k projections (transposed out [e, s]) + RoPE
                for ec in range(2):
                    for name in which:
                        w_sb = wq_sb if name == "q" else wk_sb
                        out_sb = qT if name == "q" else kT
                        esl = slice(ec * 128, (ec + 1) * 128)
                        ps = PP[0].tile([128, 512], f32, tag="ps")
                        for s2 in range(2):
                            mm3(ps, s2 * 256, w_sb, xt[name], esl, s2 * 256, 256)
                        csl = slice(sc * 512, (sc + 1) * 512)
                        # stream_shuffle cannot convert dtypes on HW: keep f32
                        sh = rp.tile([128, 512], f32, tag="sh")
                        nc.vector.stream_shuffle(sh, ps, SHUF)
                        # GPSIMD cannot touch PSUM (walrus): DVE reads PSUM,
                        # Pool handles the SBUF-only multiply and add
                        t1 = rp.tile([128, 512], f16, tag="t1")
                        nc.vector.tensor_mul(t1, ps, A_sb[:, csl])
                        t2 = rp.tile([128, 512], f16, tag="t2")
                        nc.gpsimd.tensor_mul(t2, sh, B_sb[:, csl])
                        nc.gpsimd.tensor_add(out_sb[:, ec, csl], t1, t2)

            def emit_proj_v(sc, xt):
                # v projection, natural out [s, e], into v_ext slots; two
                # 128-position blocks share one [128, 512] PSUM tile
                for u in range(2):
                    psv = PP[0].tile([128, 512], f32, tag="ps")
                    for st4 in (2 * u, 2 * u + 1):
                        out = psv[:, (st4 % 2) * 256 : (st4 % 2) * 256 + 256]
                        ssl4 = slice(st4 * 128, (st4 + 1) * 128)
                        terms = ((0, 0), (1, 0), (0, 1))  # (x plane, w plane)
                        for ti, (xp_, wp) in enumerate(terms):
                            for c in range(KC // 2):
                                wpair = wv_sb[:, wp, c].rearrange("p (u e) -> p u e", u=2)
                                nc.tensor.matmul(
                                    out,
                                    lhsT=xt["v"][:, xp_, 2 * c : 2 * c + 2, ssl4],
                                    rhs=wpair,
                                    start=(ti == 0 and c == 0),
                                    stop=(ti == 2 and c == KC // 2 - 1),
                                    perf_mode=DR,
                                )
                    nc.vector.tensor_copy(
                        v_sb[:, sc * 4 + 2 * u : sc * 4 + 2 * u + 2, :, 0:64],
                        psv.rearrange("p (s2 h d) -> p s2 h d", s2=2, h=HPC),
                    )

            def emit_attn(t, stp, op, yp, trp=None):
                """Attention + out-projection for query tile t."""
                c0 = max(0, t - 4)
                ncv = t - c0 + 1
                tsl = slice(t * 128, (t + 1) * 128)
                attn_t = sb2.tile([128, HPC, 64], f16, tag="attn")
                po = op.tile([128, HPC, 66], f32, tag="po")
                for h in range(HPC):
                    ec, hh = h // 2, h % 2
                    psl = slice(hh * 64, (hh + 1) * 64)
                    pst = stp.tile([128, 5, 128], f32, tag="st")
                    # Middle chunks first (plain start/stop groups), then each
                    # boundary chunk as an adjacent (fp8-DR mask seed, QK
                    # accumulate) pair: an intervening start=True matmul on the
                    # engine corrupts an open accumulation group.
                    bmask = {ncv - 1: maskDP_sb}
                    if t >= 4:
                        bmask[0] = maskLP_sb
                    order = [si for si in range(ncv) if si not in bmask]
                    order += sorted(bmask)
                    for si in order:
                        c = c0 + si
                        if si in bmask:
                            sd = nc.tensor.matmul(
                                pst[:, si, :], lhsT=seedI_sb, rhs=bmask[si],
                                start=True, stop=False, perf_mode=DR,
                                skip_group_check=True,
                            )
                        mm = nc.tensor.matmul(
                            pst[:, si, :],
                            lhsT=kT[psl, ec, c * 128 : (c + 1) * 128],
                            rhs=qT[psl, ec, tsl],
                            start=(si not in bmask),
                            stop=True,
                            skip_group_check=True,
                        )
                        if si in bmask:
                            add_dep_helper(
                                mm.ins, sd.ins, sync=False,
                                reason="mask preload before score accumulate",
                            )
                    pt = sb2.tile([128, 5, 128], f16, tag="pt", bufs=4)
                    nc.scalar.activation(
                        pt[:, 0:ncv, :], pst[:, 0:ncv, :], Exp,
                        scale=EXP_SCALE, bias=bias_sb,
                    )
                    for si, c in enumerate(range(c0, t + 1)):
                        nc.tensor.matmul(
                            po[:, h, 0:65],
                            lhsT=pt[:, si, :],
                            rhs=v_sb[:, c, h, :],
                            start=(si == 0),
                            stop=(si == ncv - 1),
                        )
                # normalize + transpose per head-pair so each half leaves as
                # soon as its two heads' PV finish (shortens the tail chain)
                attnT_t = sb2.tile([128, 2, 128], f16, tag="attnT")
                attn_flat = attn_t.rearrange("p h d -> p (h d)")
                rc = sb2.tile([128, HPC, 1], f32, tag="rc")
                for ec in range(2):
                    hs = slice(2 * ec, 2 * ec + 2)
                    nc.vector.reciprocal(rc[:, hs], po[:, hs, 64:65])
                    nc.vector.tensor_mul(
                        attn_t[:, hs], po[:, hs, 0:64],
                        rc[:, hs].broadcast_to([128, 2, 64])
                    )
                    if trp is None:
                        # out[p, 0, i] = attn_flat[i, ec*128 + p]
                        nc.sync.dma_start_transpose(
                            attnT_t[:, ec : ec + 1, :],
                            attn_flat[:, ec * 128 : (ec + 1) * 128],
                        )
                    else:
                        # tail: PE transpose into a spare B-phase PSUM bank
                        # avoids the ~2.4us DMA-XBAR init+sem latency
                        ptr = trp.tile([128, 128], f16, tag="tr", bufs=1)
                        nc.tensor.transpose(
                            ptr, attn_flat[:, ec * 128 : (ec + 1) * 128], ident_sb
                        )
                        nc.vector.tensor_copy(attnT_t[:, ec, :], ptr)
                a8h = sb2.tile([128, 2, 128], fp8, tag="a8h")
                nc.gpsimd.tensor_copy(a8h, attnT_t)
                a8l = sb2.tile([128, 2, 128], fp8, tag="a8l")
                nc.gpsimd.tensor_sub(a8l, attnT_t, a8h)
                y_sb = sb2.tile([128, 1024], f16, tag="ysb", bufs=3)
                for nch in range(2):
                    py = yp.tile([128, 512], f32, tag="py")
                    for n4 in range(2):
                        out = py[:, n4 * 256 : (n4 + 1) * 256]
                        nsl = slice(nch * 512 + n4 * 256, nch * 512 + (n4 + 1) * 256)
                        terms = ((a8h, 0), (a8l, 0), (a8h, 1))
                        for ti, (at, wp) in enumerate(terms):
                            nc.tensor.matmul(
                                out, lhsT=at,
                                rhs=woP_sb[:, wp, 0:2, nsl],
                                start=(ti == 0), stop=(ti == 2),
                                perf_mode=DR,
                            )
                    if t >= 12:
                        # tail: split the copy across ACT+DVE to free py fast
                        nc.scalar.copy(y_sb[:, nch * 512 : nch * 512 + 256], py[:, 0:256])
                        nc.vector.tensor_copy(
                            y_sb[:, nch * 512 + 256 : (nch + 1) * 512], py[:, 256:512])
                    else:
                        nc.vector.tensor_copy(y_sb[:, nch * 512 : (nch + 1) * 512], py)
                nc.sync.dma_start(out=y[tsl, :], in_=y_sb)

            PP = [None]
            # software pipeline: x tiles prefetched two chunks ahead; the
            # projection pieces of chunk sc interleave with the attention
            # tiles of chunk sc-1 so exp (ACT) and PE stay co-scheduled
            xts = {0: emit_dma(0)}
            # woT is unused by compute; its load occupies the same early-ACT
            # queue slot as before and keeps the downstream DMA alignment
            nc.scalar.dma_start(out=woT_sb, in_=woT[:].rearrange("(c p) n -> p c n", p=128))
            nc.scalar.dma_start(out=woP_sb, in_=woP[:].rearrange("t (c p) n -> p t c n", p=128))
            nc.scalar.dma_start(out=seedI_sb, in_=seedI[:])
            nc.scalar.dma_start(out=ident_sb, in_=ident[:])
            nc.scalar.dma_start(out=maskDP_sb, in_=maskDP[:])
            nc.scalar.dma_start(out=maskLP_sb, in_=maskLP[:])
            xts[1] = emit_dma(1)
            with tc.tile_pool(name="pst", bufs=2, space="PSUM") as stp, \
                 tc.tile_pool(name="py", bufs=1, space="PSUM") as yp:
                with tc.tile_pool(name="pp", bufs=2, space="PSUM") as pp, \
                     tc.tile_pool(name="po", bufs=1, space="PSUM") as op:
                    PP[0] = pp
                    x0 = xts.pop(0)
                    emit_proj_qk(0, x0, "q")
                    emit_proj_qk(0, x0, "k")
                    emit_proj_v(0, x0)
                    for sc in (1, 2, 3):
                        if sc + 1 <= 3:
                            xts[sc + 1] = emit_dma(sc + 1)
                        xc = xts.pop(sc)
                        t0 = 4 * (sc - 1)
                        emit_attn(t0, stp, op, yp)
                        emit_proj_qk(sc, xc, "q")
                        emit_attn(t0 + 1, stp, op, yp)
                        emit_proj_qk(sc, xc, "k")
                        emit_attn(t0 + 2, stp, op, yp)
                        emit_proj_v(sc, xc)
                        emit_attn(t0 + 3, stp, op, yp)
                # tail: po double-buffered out of the freed projection banks,
                # plus a PSUM bank for PE transposes
                with tc.tile_pool(name="poB", bufs=2, space="PSUM") as opB, \
                     tc.tile_pool(name="ptrB", bufs=1, space="PSUM") as trpB:
                    for t in range(12, 16):
                        emit_attn(t, stp, opB, yp, trpB)

    if do_compile:
        nc.compile()
    return nc


_CACHE = {}


def _get_nc():
    if "nc" not in _CACHE:
        _CACHE["nc"] = build_bass()
    return _CACHE["nc"]


def _in_maps(q, k, v, wq, wk, wv, wo):
    consts = _consts()
    perm = _head_perm()
    maps = []
    xs = {}
    for b in range(B):
        xs[b] = {
            "xq": _pack2(np.ascontiguousarray(q[b].T).astype(np.float32)),
            "xk": _pack2(np.ascontiguousarray(k[b].T).astype(np.float32)),
            "xv": _pack2(np.ascontiguousarray(v[b].T).astype(np.float32)),
        }
    for c in range(N_CORES):
        b, g = c // 4, c % 4
        esl = slice(g * E, (g + 1) * E)
        wq_c = wq[esl].astype(np.float32)[perm]
        wk_c = wk[esl].astype(np.float32)[perm]
        m = {
            "wq": _packw(np.ascontiguousarray(wq_c.T) * WSC),
            "wk": _packw(np.ascontiguousarray(wk_c.T) * WSC),
            "wv": _packw(np.ascontiguousarray(wv[esl].astype(np.float32).T) * WSC),
            "woT": np.ascontiguousarray(wo[:, esl].T),
            "woP": _pack2(np.ascontiguousarray(wo[:, esl].astype(np.float32).T) * WSC),
        }
        m.update(xs[b])
        m.update(consts)
        maps.append(m)
    return maps


def kernel(q, k, v, wq, wk, wv, wo):
    q, k, v = (np.asarray(a, dtype=np.float16) for a in (q, k, v))
    wq, wk, wv, wo = (np.asarray(a, dtype=np.float16) for a in (wq, wk, wv, wo))
    from concourse.bass_utils import run_bass_kernel_spmd

    nc = _get_nc()
    maps = _in_maps(q, k, v, wq, wk, wv, wo)
    res = run_bass_kernel_spmd(nc, maps, core_ids=list(range(N_CORES)))
    out = np.zeros((B, S, DIM), dtype=np.float32)
    for c in range(N_CORES):
        out[c // 4] += np.asarray(res.results[c]["y"]).astype(np.float32)
    # the fp8 out-projection carries the x16 v-scale and the x16 wo-scale
    out *= 1.0 / (WSC * WSC)
    return out.astype(np.float16)
